# revision 12
# baseline (speedup 1.0000x reference)
"""Trainium2 Bass kernel for 16-head MHA (B=2, S=4096, D=1024).

Sharding: 8 cores = 2 batches x 4 head-groups (4 heads each).
Each core computes, for its (batch b, head group g):
    Q^T/K^T ([256, S] in head-major layout), V ([S, 256] + ones cols),
    per head: S^T = K Q^T (scores transposed), P = exp(S^T/8),
    [O^T; D] = [V|1]^T @ P^T  (PV matmul with fused denominator row),
    O^T_norm = O^T / D, Y^T_partial = woT^T @ O^T_norm.
Host sums the 4 per-head-group partials per batch and adds b_o.

All inputs arrive host-side pre-transposed so every DMA is contiguous.
"""

import os
import sys

sys.path.insert(0, "/opt/trn_rl_repo")
os.environ.setdefault("MYCRO_LOCAL_CACHE", "1")

from contextlib import ExitStack

import numpy as np

import concourse.bass as bass
import concourse.tile as tile
from concourse import bacc, mybir

F32 = mybir.dt.float32
F32R = mybir.dt.float32r
BF16 = mybir.dt.bfloat16
AF = mybir.ActivationFunctionType
ALU = mybir.AluOpType

D = 1024  # d_model
NH = 16  # total heads
DH = 64  # head dim
HPC = 4  # heads per core
MG = HPC * DH  # 256 model cols per core


def build_module(S: int = 4096, dbg: bool = False) -> bass.Bass:
    nc = bacc.Bacc("TRN2", target_bir_lowering=False, debug=False, num_devices=8)

    xq = nc.dram_tensor("xqt", [D, S], F32, kind="ExternalInput")  # q[b].T
    xk = nc.dram_tensor("xkt", [D, S], F32, kind="ExternalInput")
    xv = nc.dram_tensor("xvt", [D, S], F32, kind="ExternalInput")
    wq = nc.dram_tensor("wqt", [D, MG], F32, kind="ExternalInput")  # w_q[rows_g].T
    wk = nc.dram_tensor("wkt", [D, MG], F32, kind="ExternalInput")
    wv = nc.dram_tensor("wvt", [D, MG], F32, kind="ExternalInput")
    wo = nc.dram_tensor("wot", [MG, D], F32, kind="ExternalInput")  # w_o[:, cols_g].T
    bq = nc.dram_tensor("bq", [MG], F32, kind="ExternalInput")
    bk = nc.dram_tensor("bk", [MG], F32, kind="ExternalInput")
    bv = nc.dram_tensor("bv", [MG], F32, kind="ExternalInput")
    yt = nc.dram_tensor("yt", [D, S], F32, kind="ExternalOutput")  # partial y[b].T
    if dbg:
        dqt = [nc.dram_tensor(f"dqt{i}", [128, S], BF16, kind="ExternalOutput") for i in range(2)]
        dkt = [nc.dram_tensor(f"dkt{i}", [128, S], BF16, kind="ExternalOutput") for i in range(2)]
        dot = [nc.dram_tensor(f"dot{i}", [128, S], BF16, kind="ExternalOutput") for i in range(2)]
        dvs = nc.dram_tensor("dvs", [128, (S // 128) * HPC * (DH + 1)], BF16, kind="ExternalOutput")
        dpt = [nc.dram_tensor(f"dpt{i}", [128, min(1024, S)], BF16, kind="ExternalOutput") for i in range(2)]
        dpv = [nc.dram_tensor(f"dpv{i}", [DH + 1, min(1024, S)], F32, kind="ExternalOutput") for i in range(2)]
        drd = [nc.dram_tensor(f"drd{i}", [1, min(1024, S)], F32, kind="ExternalOutput") for i in range(2)]
        drdb = [nc.dram_tensor(f"drdb{i}", [DH, min(1024, S)], F32, kind="ExternalOutput") for i in range(2)]

    SC = min(1024, S)  # attention s-chunk width
    n_sc = S // SC
    n_tc = S // 128  # key/value chunks of 128
    PSC = min(512, S)  # projection s-chunk
    n_psc = S // PSC
    VTG = min(1024, S)  # v-projection t group width
    n_vtg = S // VTG

    with tile.TileContext(nc) as tc, ExitStack() as ctx:
        persist = ctx.enter_context(tc.tile_pool(name="persist", bufs=1))

        # -------- weights / biases to SBUF --------
        wq_s = persist.tile([128, D // 128, MG], BF16, tag="wq")
        wk_s = persist.tile([128, D // 128, MG], BF16, tag="wk")
        wv_s = persist.tile([128, D // 128, MG], BF16, tag="wv")
        wo_s = persist.tile([128, MG // 128, D], BF16, tag="wo")
        bq_r = persist.tile([1, MG], BF16, tag="bq")
        bk_r = persist.tile([1, MG], BF16, tag="bk")
        bv_r = persist.tile([1, MG], BF16, tag="bv")
        ones_r = persist.tile([1, PSC], BF16, tag="ones_r")
        nc.gpsimd.dma_start(wq_s[:], wq[:].rearrange("(d p) m -> p d m", p=128))
        nc.gpsimd.dma_start(wk_s[:], wk[:].rearrange("(d p) m -> p d m", p=128))
        nc.gpsimd.dma_start(wv_s[:], wv[:].rearrange("(d p) m -> p d m", p=128))
        # cast f32 -> bf16 during DMA (SWDGE)
        nc.gpsimd.dma_start(wo_s[:], wo[:].rearrange("(t p) n -> p t n", p=128))
        nc.gpsimd.dma_start(bq_r[:], bq[:].unsqueeze(0))
        nc.gpsimd.dma_start(bk_r[:], bk[:].unsqueeze(0))
        nc.gpsimd.dma_start(bv_r[:], bv[:].unsqueeze(0))
        nc.vector.memset(ones_r[:], 1.0)

        # -------- persistent activations --------
        # Q^T/K^T per head-pair: [128, S] (partitions = 2 heads x 64)
        qt = [persist.tile([128, S], BF16, tag=f"qt{i}", name=f"qt{i}") for i in range(2)]
        kt = [persist.tile([128, S], BF16, tag=f"kt{i}", name=f"kt{i}") for i in range(2)]
        # V: [t=128, tc, 4*(64+1)] bf16; col 64 of each head's 65-group = ones
        vs = persist.tile([128, n_tc, HPC * (DH + 1)], BF16, tag="vs")
        # O^T per m-chunk (= head pair): [128, S] bf16
        ot = [persist.tile([128, S], BF16, tag=f"ot{i}", name=f"ot{i}") for i in range(2)]

        for h in range(HPC):
            nc.vector.memset(vs[:, :, h * 65 + 64 : h * 65 + 65], 1.0)

        # -------- phase 1: Q^T / K^T projections --------
        with tc.tile_pool(name="qk_stage", bufs=10) as stage, tc.tile_pool(
            name="proj_psum", bufs=4, space="PSUM"
        ) as pp:
            for si in range(n_psc):
                xq_t = []
                xk_t = []
                for d in range(D // 128):
                    t1 = stage.tile([128, PSC], BF16, tag="xq", name="xq")
                    nc.gpsimd.dma_start(
                        t1[:], xq[d * 128 : (d + 1) * 128, si * PSC : (si + 1) * PSC]
                    )
                    xq_t.append(t1)
                    t2 = stage.tile([128, PSC], BF16, tag="xk", name="xk")
                    nc.gpsimd.dma_start(
                        t2[:], xk[d * 128 : (d + 1) * 128, si * PSC : (si + 1) * PSC]
                    )
                    xk_t.append(t2)
                for mc in range(MG // 128):
                    ps = pp.tile([128, PSC], F32, tag="pjq")
                    for d in range(D // 128):
                        nc.tensor.matmul(
                            ps[:],
                            wq_s[:, d, mc * 128 : (mc + 1) * 128],
                            xq_t[d][:],
                            start=(d == 0),
                            stop=False,
                        )
                    nc.tensor.matmul(
                        ps[:],
                        bq_r[0:1, mc * 128 : (mc + 1) * 128],
                        ones_r[0:1, :],
                        start=False,
                        stop=True,
                    )
                    nc.vector.tensor_copy(
                        qt[mc][:, si * PSC : (si + 1) * PSC], ps[:]
                    )
                    ps2 = pp.tile([128, PSC], F32, tag="pjk")
                    for d in range(D // 128):
                        nc.tensor.matmul(
                            ps2[:],
                            wk_s[:, d, mc * 128 : (mc + 1) * 128],
                            xk_t[d][:],
                            start=(d == 0),
                            stop=False,
                        )
                    nc.tensor.matmul(
                        ps2[:],
                        bk_r[0:1, mc * 128 : (mc + 1) * 128],
                        ones_r[0:1, :],
                        start=False,
                        stop=True,
                    )
                    nc.vector.tensor_copy(
                        kt[mc][:, si * PSC : (si + 1) * PSC], ps2[:]
                    )

        # -------- phase 2: V projection (natural [t, m] layout) --------
        with tc.tile_pool(name="v_stage", bufs=10) as stage, tc.tile_pool(
            name="v_psum", bufs=4, space="PSUM"
        ) as pp:
            for tg in range(n_vtg):
                xv_t = []
                for d in range(D // 128):
                    t1 = stage.tile([128, VTG], BF16, tag="xv", name="xv")
                    nc.gpsimd.dma_start(
                        t1[:], xv[d * 128 : (d + 1) * 128, tg * VTG : (tg + 1) * VTG]
                    )
                    xv_t.append(t1)
                for tl in range(VTG // 128):
                    ps = pp.tile([128, MG], F32, tag="pjv")
                    for d in range(D // 128):
                        nc.tensor.matmul(
                            ps[:],
                            xv_t[d][:, tl * 128 : (tl + 1) * 128],
                            wv_s[:, d, :],
                            start=(d == 0),
                            stop=False,
                        )
                    nc.tensor.matmul(
                        ps[:],
                        ones_r[0:1, 0:128],
                        bv_r[0:1, :],
                        start=False,
                        stop=True,
                    )
                    tcix = tg * (VTG // 128) + tl
                    for h in range(HPC):
                        nc.vector.tensor_copy(
                            vs[:, tcix, h * 65 : h * 65 + 64],
                            ps[:, h * DH : (h + 1) * DH],
                        )

        # -------- phase 3: attention --------
        with tc.tile_pool(name="qk_psum", bufs=2, space="PSUM") as qkp, tc.tile_pool(
            name="pv_psum", bufs=2, space="PSUM"
        ) as pvp, tc.tile_pool(name="pt_pool", bufs=4) as ptp, tc.tile_pool(
            name="norm", bufs=4
        ) as normp:
            for hp in range(2):  # head pair (m-chunk)
                for si in range(n_sc):
                    pv = [pvp.tile([DH + 1, SC], F32, tag="pv", name="pv") for _ in range(2)]
                    for tcix in range(n_tc):
                        for hh in range(2):  # head within pair
                            po = DH * hh
                            qk = qkp.tile([128, SC], F32, tag="qk")
                            for nn in range(SC // 512):
                                nc.tensor.matmul(
                                    qk[:, nn * 512 : (nn + 1) * 512],
                                    kt[hp][po : po + DH, tcix * 128 : (tcix + 1) * 128],
                                    qt[hp][
                                        po : po + DH,
                                        si * SC + nn * 512 : si * SC + (nn + 1) * 512,
                                    ],
                                    start=True,
                                    stop=True,
                                )
                            pt = ptp.tile([128, SC], BF16, tag="pt")
                            nc.scalar.activation(pt[:], qk[:], AF.Exp, scale=0.125)
                            if dbg and hp == 0 and si == 0 and tcix == 0:
                                nc.sync.dma_start(dpt[hh][:], pt[:])
                            h = hp * 2 + hh
                            for nn in range(SC // 512):
                                nc.tensor.matmul(
                                    pv[hh][:, nn * 512 : (nn + 1) * 512],
                                    vs[:, tcix, h * 65 : (h + 1) * 65],
                                    pt[:, nn * 512 : (nn + 1) * 512],
                                    start=(tcix == 0),
                                    stop=(tcix == n_tc - 1),
                                )
                    for hh in range(2):
                        if dbg and hp == 0 and si == 0:
                            dbg_pv = normp.tile([DH + 1, SC], F32, tag="dbgpv", name="dbgpv")
                            nc.vector.tensor_copy(dbg_pv[:], pv[hh][:])
                            nc.sync.dma_start(dpv[hh][:], dbg_pv[:])
                        po = DH * hh
                        dsb = normp.tile([1, SC], F32, tag="dsb", name="dsb")
                        nc.vector.tensor_copy(dsb[:], pv[hh][DH : DH + 1, :])
                        rd = normp.tile([1, SC], F32, tag="rd", name="rd")
                        nc.vector.reciprocal_approx_fast(rd[:], dsb[:])
                        rdb = normp.tile([DH, SC], F32, tag="rdb", name="rdb")
                        nc.gpsimd.partition_broadcast(rdb[:], rd[:])
                        if dbg and hp == 0 and si == 0:
                            nc.sync.dma_start(drd[hh][:], rd[:])
                            nc.sync.dma_start(drdb[hh][:], rdb[:])
                        dst = ot[hp][po : po + DH, si * SC : (si + 1) * SC]
                        nc.vector.tensor_tensor(dst, pv[hh][0:DH, :], rdb[:], ALU.mult)

        if dbg:
            for i in range(2):
                nc.sync.dma_start(dqt[i][:], qt[i][:])
                nc.sync.dma_start(dkt[i][:], kt[i][:])
                nc.sync.dma_start(dot[i][:], ot[i][:])
            nc.sync.dma_start(dvs[:], vs[:].rearrange("p a b -> p (a b)"))

        # -------- phase 4: output projection --------
        with tc.tile_pool(name="op_psum", bufs=4, space="PSUM") as opp, tc.tile_pool(
            name="y_stage", bufs=4
        ) as ysp:
            for si in range(n_psc):
                for nn8 in range(D // 128):
                    ps = opp.tile([128, PSC], F32, tag="yp")
                    for mt in range(MG // 128):
                        nc.tensor.matmul(
                            ps[:],
                            wo_s[:, mt, nn8 * 128 : (nn8 + 1) * 128],
                            ot[mt][:, si * PSC : (si + 1) * PSC],
                            start=(mt == 0),
                            stop=(mt == MG // 128 - 1),
                        )
                    ys = ysp.tile([128, PSC], F32, tag="ys")
                    nc.vector.tensor_copy(ys[:], ps[:])
                    nc.sync.dma_start(
                        yt[nn8 * 128 : (nn8 + 1) * 128, si * PSC : (si + 1) * PSC],
                        ys[:],
                    )

    nc.compile()
    return nc


_MODULE_CACHE: dict = {}


def _get_module(S: int) -> bass.Bass:
    if S not in _MODULE_CACHE:
        _MODULE_CACHE[S] = build_module(S)
    return _MODULE_CACHE[S]


def make_in_maps(q, k, v, w_q, b_q, w_k, b_k, w_v, b_v, w_o, b_o):
    """Shard full inputs into 8 per-core input maps (host-side prep)."""
    f = lambda a: np.ascontiguousarray(np.asarray(a, dtype=np.float32))
    q, k, v = f(q), f(k), f(v)
    w_q, w_k, w_v, w_o = f(w_q), f(w_k), f(w_v), f(w_o)
    b_q, b_k, b_v = f(b_q), f(b_k), f(b_v)
    in_maps = []
    for core in range(8):
        b, g = core // 4, core % 4
        rows = slice(g * MG, (g + 1) * MG)
        in_maps.append(
            {
                "xqt": np.ascontiguousarray(q[b].T),
                "xkt": np.ascontiguousarray(k[b].T),
                "xvt": np.ascontiguousarray(v[b].T),
                "wqt": np.ascontiguousarray(w_q[rows].T),
                "wkt": np.ascontiguousarray(w_k[rows].T),
                "wvt": np.ascontiguousarray(w_v[rows].T),
                "wot": np.ascontiguousarray(w_o[:, rows].T),
                "bq": np.ascontiguousarray(b_q[rows]),
                "bk": np.ascontiguousarray(b_k[rows]),
                "bv": np.ascontiguousarray(b_v[rows]),
            }
        )
    return in_maps


def gather_output(results, b_o, B, S):
    y = np.zeros((B, S, D), np.float32)
    for core in range(8):
        b = core // 4
        y[b] += results[core]["yt"].T
    y += np.asarray(b_o, np.float32)[None, None, :]
    return y


def run(inputs: dict, trace: bool = False):
    """Run on 8 NeuronCores; returns (y, BassKernelResults)."""
    from concourse import bass_utils

    B, S, _ = np.asarray(inputs["q"]).shape
    mod = _get_module(S)
    in_maps = make_in_maps(**inputs)
    res = bass_utils.run_bass_kernel_spmd(
        mod, in_maps, core_ids=list(range(8)), trace=trace
    )
    y = gather_output(res.results, inputs["b_o"], B, S)
    return y, res


def kernel(q, k, v, w_q, b_q, w_k, b_k, w_v, b_v, w_o, b_o):
    y, _ = run(
        dict(
            q=q, k=k, v=v, w_q=w_q, b_q=b_q, w_k=w_k, b_k=b_k,
            w_v=w_v, b_v=b_v, w_o=w_o, b_o=b_o,
        )
    )
    return y
